# revision 18
# baseline (speedup 1.0000x reference)
"""EMA head kernel for Trainium2 (Bass/Tile), 8 NeuronCores.

Problem: alpha = clip(sigmoid(MLP(feat)), 0.01, 0.99) per (t, b);
         y[0] = r[0]; y[t] = (1-alpha[t])*y[t-1] + alpha[t]*r[t].

Sharding: time dim T=4096 split into 8 slabs of 512 (all B=256 per core).
Each core computes, for its slab, the local affine-scan pieces
    z[t] = A[t]*z[t-1] + Bv[t]   (z[-1] = 0),   A = 1-alpha, Bv = alpha*r
    P[t] = A[t]*P[t-1]           (P[-1] = 1)
and the host stitches slabs with   y = z + P * carry,  carry' = y[-1].
carry_0 = r[0] reproduces y[0] = r[0] exactly: a*r + (1-a)*r = r.

v9: feat pre-cast to fp16 on host (the MLP runs in fp16 anyway),
halving HBM traffic, loaded via the SYNC HWDGE queue exclusively
(constants and r ride the SCALAR HWDGE queue so feat prefetch is never
blocked), TIME on partitions ([t=128, b=64 * f=128] tiles, 16 KB
contiguous per partition line).  Per batch element: PE transpose
[t,f] -> [f,t] (8 per 2 KB PSUM bank), copy PSUM->SBUF (DVE/ACT
alternating, plain fp16), matmul lhsT=ftT rhs=W1 -> h [t, 16]
collected 32 b-slots per PSUM bank, drain +b1 (DVE, fused PSUM read) /
relu (ACT) / *W2 (GPSIMD) / reduce (DVE) -> apre [t=128, b].  apre is
PE-transposed to [b=128, t] per t-chunk, then sigmoid/clip/A/Bv and
the two tensor_tensor_scans run whole-row at the end.  r arrives
pre-transposed [b, t] from the host.
"""

import numpy as np

T, B, FEAT, HID = 4096, 256, 128, 16
NCORES = 8
TLOC = T // NCORES  # 512
NH = 2              # batch halves of 128 (contiguous: b = h*128 + p)
NTC = TLOC // 128   # 4 t-chunks of 128 partitions
BQ = 64             # batch elems per feat DMA (16 KB/partition chunk)
NBQ = B // BQ       # 4

_CACHE = {}


def _build_program():
    import concourse.bacc as bacc
    import concourse.bass as bass
    import concourse.tile as tile
    from concourse import mybir
    from concourse.masks import make_identity

    fp32 = mybir.dt.float32
    fp16 = mybir.dt.float16
    AF = mybir.ActivationFunctionType
    OP = mybir.AluOpType

    nc = bacc.Bacc("TRN2", target_bir_lowering=False, debug=False,
                   num_devices=NCORES)

    feat_d = nc.dram_tensor("feat", [TLOC, B, FEAT], fp16, kind="ExternalInput")
    rt_d = nc.dram_tensor("rt", [B, TLOC], fp32, kind="ExternalInput")
    w1_d = nc.dram_tensor("w1", [FEAT, HID], fp16, kind="ExternalInput")
    b1_d = nc.dram_tensor("b1rep", [128, 32, HID], fp32, kind="ExternalInput")
    w2_d = nc.dram_tensor("w2rep", [128, 32, HID], fp32, kind="ExternalInput")
    b2_d = nc.dram_tensor("b2col", [128, 1], fp32, kind="ExternalInput")
    z_d = nc.dram_tensor("z", [NH, 128, TLOC], fp32, kind="ExternalOutput")
    p_d = nc.dram_tensor("p", [NH, 128, TLOC], fp32, kind="ExternalOutput")

    with tile.TileContext(nc) as tc:
        with (
            tc.tile_pool(name="singles", bufs=1) as singles,
            tc.tile_pool(name="featin", bufs=4) as featin,
            tc.tile_pool(name="ftps", bufs=3, space="PSUM") as ftps,
            tc.tile_pool(name="hps", bufs=2, space="PSUM") as hps,
            tc.tile_pool(name="ftsb", bufs=3) as ftsb,
            tc.tile_pool(name="hwork", bufs=2) as hwork,
        ):
            # ------- constants (scalar HWDGE queue; sync = feat only) -------
            ident = singles.tile([128, 128], fp16)
            make_identity(nc, ident)
            ident32 = singles.tile([128, 128], fp32)
            make_identity(nc, ident32)
            w1_sb = singles.tile([128, HID], fp16)
            nc.scalar.dma_start(w1_sb, w1_d[:, :])
            b1rep = singles.tile([128, 32, HID], fp32)
            nc.scalar.dma_start(b1rep, b1_d[:, :, :])
            w2rep = singles.tile([128, 32, HID], fp32)
            nc.scalar.dma_start(w2rep, w2_d[:, :, :])
            b2col = singles.tile([128, 1], fp32)
            nc.scalar.dma_start(b2col, b2_d[:, :])
            ones_sb = singles.tile([128, TLOC], fp32)
            nc.vector.memset(ones_sb, 1.0)

            # ---- r (pre-transposed on host): rT [b, t] per half ----
            rT = [singles.tile([128, TLOC], fp32, tag=f"rT{h}", name=f"rT{h}")
                  for h in range(NH)]
            for h in range(NH):
                nc.scalar.dma_start(rT[h], rt_d[h * 128:(h + 1) * 128, :])

            # per-tchunk alpha_pre accumulators [128 t, B]
            apre = [singles.tile([128, B], fp32, tag=f"apre{tc_}",
                                 name=f"apre{tc_}")
                    for tc_ in range(NTC)]
            # transposed alpha_pre [128 b, t] per half
            apreT = [singles.tile([128, TLOC], fp32, tag=f"apreT{h}",
                                  name=f"apreT{h}")
                     for h in range(NH)]

            # ---------------- main feat pipeline ----------------
            copy_parity = 0
            for tcnk in range(NTC):
                if tcnk == 0:
                    # small leading tiles so PE work starts ASAP
                    chunks = [(0, 32), (32, 32), (64, 64), (128, 64),
                              (192, 64)]
                else:
                    chunks = [(0, 64), (64, 64), (128, 64), (192, 64)]
                for cb0, cbn in chunks:
                    ft = featin.tile([128, BQ * FEAT], fp16, tag="ft")
                    nc.sync.dma_start(
                        ft[:, :cbn * FEAT],
                        feat_d[tcnk * 128:(tcnk + 1) * 128,
                               cb0:cb0 + cbn, :].rearrange(
                                   "t b f -> t (b f)"))

                    for g in range(cbn // 32):  # 32-b granule of this chunk
                        hbank = hps.tile([128, 32, HID], fp32, tag="hbank")
                        for q in range(0, 32, 8):
                            ftp = ftps.tile([128, 8, 128], fp16, tag="ftp16")
                            for s in range(8):
                                bl = g * 32 + q + s
                                nc.tensor.transpose(
                                    ftp[:, s, :],
                                    ft[:, bl * FEAT:(bl + 1) * FEAT], ident)
                            fts = ftsb.tile([128, 8, 128], fp16, tag="fts")
                            if copy_parity == 0:
                                nc.vector.tensor_copy(fts, ftp)
                            else:
                                nc.scalar.copy(fts, ftp)
                            copy_parity ^= 1
                            for s in range(8):
                                nc.tensor.matmul(hbank[:, q + s, :],
                                                 fts[:, s, :], w1_sb)

                        # drain this 32-b bank -> apre columns
                        b0 = cb0 + g * 32
                        hb = hwork.tile([128, 32, HID], fp32, tag="hb")
                        nc.vector.tensor_add(hb, hbank, b1rep)
                        hrelu = hwork.tile([128, 32, HID], fp32, tag="hrelu")
                        nc.scalar.activation(hrelu, hb, AF.Relu)
                        hw = hwork.tile([128, 32, HID], fp32, tag="hw")
                        nc.gpsimd.tensor_mul(hw, hrelu, w2rep)
                        nc.vector.tensor_reduce(
                            apre[tcnk][:, b0:b0 + 32],
                            hw, axis=mybir.AxisListType.X, op=OP.add)

                # all of apre[tcnk] done: transpose [t, b] -> [b, t] halves
                aps = ftps.tile([128, 4, 128], fp32, tag="aps")
                for h in range(NH):
                    nc.tensor.transpose(
                        aps[:, h, :],
                        apre[tcnk][:, h * 128:(h + 1) * 128], ident32)
                for h in range(NH):
                    nc.scalar.copy(
                        apreT[h][:, tcnk * 128:(tcnk + 1) * 128], aps[:, h, :])

            # ---------------- alpha -> scans -> out ----------------
            for h in range(NH):
                alpha = singles.tile([128, TLOC], fp32, tag=f"alpha{h}",
                                     name=f"alpha{h}")
                nc.scalar.activation(alpha, apreT[h], AF.Sigmoid, bias=b2col)
                nc.vector.tensor_scalar(alpha, alpha, 0.01, 0.99,
                                        op0=OP.max, op1=OP.min)
                A_sb = singles.tile([128, TLOC], fp32, tag=f"A{h}",
                                    name=f"A{h}")
                nc.vector.tensor_scalar(A_sb, alpha, -1.0, 1.0,
                                        op0=OP.mult, op1=OP.add)
                Bv = singles.tile([128, TLOC], fp32, tag=f"Bv{h}",
                                  name=f"Bv{h}")
                nc.vector.tensor_mul(Bv, alpha, rT[h])
                z_sb = singles.tile([128, TLOC], fp32, tag=f"z{h}",
                                    name=f"z{h}")
                nc.vector.tensor_tensor_scan(z_sb, A_sb, Bv, 0.0,
                                             op0=OP.mult, op1=OP.add)
                p_sb = singles.tile([128, TLOC], fp32, tag=f"p{h}",
                                    name=f"p{h}")
                nc.vector.tensor_tensor_scan(p_sb, A_sb, ones_sb, 1.0,
                                             op0=OP.mult, op1=OP.mult)
                nc.sync.dma_start(z_d[h], z_sb)
                nc.sync.dma_start(p_d[h], p_sb)

    nc.finalize()
    return nc


def _get_program():
    if "nc" not in _CACHE:
        _CACHE["nc"] = _build_program()
    return _CACHE["nc"]


def kernel(r, feat, W1, b1, W2, b2, _run_kwargs=None, _return_results=False):
    from concourse.bass_utils import run_bass_kernel_spmd

    r = np.asarray(r, dtype=np.float32)
    feat16 = np.asarray(feat, dtype=np.float16)
    W1 = np.asarray(W1, dtype=np.float16)
    b1rep = np.ascontiguousarray(np.broadcast_to(
        np.asarray(b1, dtype=np.float32).reshape(1, 1, HID), (128, 32, HID)))
    w2rep = np.ascontiguousarray(np.broadcast_to(
        np.asarray(W2, dtype=np.float32).reshape(1, 1, HID), (128, 32, HID)))
    b2col = np.ascontiguousarray(np.broadcast_to(
        np.asarray(b2, dtype=np.float32).reshape(1, 1), (128, 1)))

    nc = _get_program()
    in_maps = []
    for c in range(NCORES):
        in_maps.append({
            "feat": np.ascontiguousarray(feat16[c * TLOC:(c + 1) * TLOC]),
            "rt": np.ascontiguousarray(r[c * TLOC:(c + 1) * TLOC, :, 0].T),
            "w1": W1, "b1rep": b1rep, "w2rep": w2rep, "b2col": b2col,
        })

    kw = _run_kwargs or {}
    res = run_bass_kernel_spmd(nc, in_maps, core_ids=list(range(NCORES)), **kw)

    # host stitch: y = z + P*carry per slab, carry chain across slabs
    # z/p layout: [h, p, t] with b = h*128 + p (contiguous halves)
    y = np.empty((T, B), dtype=np.float32)
    carry = r[0, :, 0].astype(np.float32)
    for c in range(NCORES):
        zc = res.results[c]["z"].reshape(B, TLOC).T
        pc = res.results[c]["p"].reshape(B, TLOC).T
        y_slab = zc + pc * carry[None, :]
        carry = y_slab[-1]
        y[c * TLOC:(c + 1) * TLOC] = y_slab
    out = y[:, :, None]
    if _return_results:
        return out, res
    return out


# revision 21
# speedup vs baseline: 1.0211x; 1.0211x over previous
"""EMA head kernel for Trainium2 (Bass/Tile), 8 NeuronCores.

Problem: alpha = clip(sigmoid(MLP(feat)), 0.01, 0.99) per (t, b);
         y[0] = r[0]; y[t] = (1-alpha[t])*y[t-1] + alpha[t]*r[t].

Sharding: time dim T=4096 split into 8 slabs of 512 (all B=256 per core).
Each core computes, for its slab, the local affine-scan pieces
    z[t] = A[t]*z[t-1] + Bv[t]   (z[-1] = 0),   A = 1-alpha, Bv = alpha*r
    P[t] = A[t]*P[t-1]           (P[-1] = 1)
and the host stitches slabs with   y = z + P * carry,  carry' = y[-1].
carry_0 = r[0] reproduces y[0] = r[0] exactly: a*r + (1-a)*r = r.

v9: feat pre-cast to fp16 on host (the MLP runs in fp16 anyway),
halving HBM traffic, loaded via the SYNC HWDGE queue exclusively
(constants and r ride the SCALAR HWDGE queue so feat prefetch is never
blocked), TIME on partitions ([t=128, b=64 * f=128] tiles, 16 KB
contiguous per partition line).  Per batch element: PE transpose
[t,f] -> [f,t] (8 per 2 KB PSUM bank), copy PSUM->SBUF (DVE/ACT
alternating, plain fp16), matmul lhsT=ftT rhs=W1 -> h [t, 16]
collected 32 b-slots per PSUM bank, drain +b1 (DVE, fused PSUM read) /
relu (ACT) / *W2 (GPSIMD) / reduce (DVE) -> apre [t=128, b].  apre is
PE-transposed to [b=128, t] per t-chunk, then sigmoid/clip/A/Bv and
the two tensor_tensor_scans run whole-row at the end.  r arrives
pre-transposed [b, t] from the host.
"""

import numpy as np

T, B, FEAT, HID = 4096, 256, 128, 16
NCORES = 8
TLOC = T // NCORES  # 512
NH = 2              # batch halves of 128 (contiguous: b = h*128 + p)
NTC = TLOC // 128   # 4 t-chunks of 128 partitions
BQ = 64             # batch elems per feat DMA (16 KB/partition chunk)
NBQ = B // BQ       # 4

_CACHE = {}


def _build_program():
    import concourse.bacc as bacc
    import concourse.bass as bass
    import concourse.tile as tile
    from concourse import mybir
    from concourse.masks import make_identity

    fp32 = mybir.dt.float32
    fp16 = mybir.dt.float16
    AF = mybir.ActivationFunctionType
    OP = mybir.AluOpType

    nc = bacc.Bacc("TRN2", target_bir_lowering=False, debug=False,
                   num_devices=NCORES)

    feat_d = nc.dram_tensor("feat", [TLOC, B, FEAT], fp16, kind="ExternalInput")
    rt_d = nc.dram_tensor("rt", [B, TLOC], fp32, kind="ExternalInput")
    w1_d = nc.dram_tensor("w1", [FEAT, HID], fp16, kind="ExternalInput")
    b1_d = nc.dram_tensor("b1rep", [128, 32, HID], fp32, kind="ExternalInput")
    w2_d = nc.dram_tensor("w2rep", [128, 32, HID], fp32, kind="ExternalInput")
    b2_d = nc.dram_tensor("b2col", [128, 1], fp32, kind="ExternalInput")
    z_d = nc.dram_tensor("z", [NH, 128, TLOC], fp32, kind="ExternalOutput")
    p_d = nc.dram_tensor("p", [NH, 128, TLOC], fp32, kind="ExternalOutput")

    with tile.TileContext(nc) as tc:
        with (
            tc.tile_pool(name="singles", bufs=1) as singles,
            tc.tile_pool(name="featin", bufs=3) as featin,
            tc.tile_pool(name="ftps", bufs=3, space="PSUM") as ftps,
            tc.tile_pool(name="hps", bufs=2, space="PSUM") as hps,
            tc.tile_pool(name="ftsb", bufs=3) as ftsb,
            tc.tile_pool(name="hwork", bufs=2) as hwork,
        ):
            # ------- constants (scalar HWDGE queue; sync = feat only) -------
            ident = singles.tile([128, 128], fp16)
            make_identity(nc, ident)
            ident32 = singles.tile([128, 128], fp32)
            make_identity(nc, ident32)
            w1_sb = singles.tile([128, HID], fp16)
            nc.scalar.dma_start(w1_sb, w1_d[:, :])
            b1rep = singles.tile([128, 32, HID], fp32)
            nc.scalar.dma_start(b1rep, b1_d[:, :, :])
            w2rep = singles.tile([128, 32, HID], fp32)
            nc.scalar.dma_start(w2rep, w2_d[:, :, :])
            b2col = singles.tile([128, 1], fp32)
            nc.scalar.dma_start(b2col, b2_d[:, :])
            ones_sb = singles.tile([128, TLOC], fp32)
            nc.vector.memset(ones_sb, 1.0)

            # ---- r (pre-transposed on host): rT [b, t] per half ----
            rT = [singles.tile([128, TLOC], fp32, tag=f"rT{h}", name=f"rT{h}")
                  for h in range(NH)]
            for h in range(NH):
                nc.scalar.dma_start(rT[h], rt_d[h * 128:(h + 1) * 128, :])

            # per-tchunk alpha_pre accumulators [128 t, B]
            apre = [singles.tile([128, B], fp32, tag=f"apre{tc_}",
                                 name=f"apre{tc_}")
                    for tc_ in range(NTC)]
            # transposed alpha_pre [128 b, t] per half
            apreT = [singles.tile([128, TLOC], fp32, tag=f"apreT{h}",
                                  name=f"apreT{h}")
                     for h in range(NH)]

            # ---------------- main feat pipeline ----------------
            copy_parity = 0
            for tcnk in range(NTC):
                chunks = [(0, 64), (64, 64), (128, 64), (192, 64)]
                for cb0, cbn in chunks:
                    ft = featin.tile([128, BQ * FEAT], fp16, tag="ft")
                    nc.sync.dma_start(
                        ft[:, :cbn * FEAT],
                        feat_d[tcnk * 128:(tcnk + 1) * 128,
                               cb0:cb0 + cbn, :].rearrange(
                                   "t b f -> t (b f)"))

                    for g in range(cbn // 32):  # 32-b granule of this chunk
                        hbank = hps.tile([128, 32, HID], fp32, tag="hbank")
                        for q in range(0, 32, 8):
                            ftp = ftps.tile([128, 8, 128], fp16, tag="ftp16")
                            for s in range(8):
                                bl = g * 32 + q + s
                                nc.tensor.transpose(
                                    ftp[:, s, :],
                                    ft[:, bl * FEAT:(bl + 1) * FEAT], ident)
                            fts = ftsb.tile([128, 8, 128], fp16, tag="fts")
                            if copy_parity == 0:
                                nc.vector.tensor_copy(fts, ftp)
                            else:
                                nc.scalar.copy(fts, ftp)
                            copy_parity ^= 1
                            for s in range(8):
                                nc.tensor.matmul(hbank[:, q + s, :],
                                                 fts[:, s, :], w1_sb)

                        # drain this 32-b bank -> apre columns
                        b0 = cb0 + g * 32
                        hb = hwork.tile([128, 32, HID], fp32, tag="hb")
                        nc.vector.tensor_add(hb, hbank, b1rep)
                        hrelu = hwork.tile([128, 32, HID], fp32, tag="hrelu")
                        nc.scalar.activation(hrelu, hb, AF.Relu)
                        hw = hwork.tile([128, 32, HID], fp32, tag="hw")
                        nc.gpsimd.tensor_mul(hw, hrelu, w2rep)
                        nc.vector.tensor_reduce(
                            apre[tcnk][:, b0:b0 + 32],
                            hw, axis=mybir.AxisListType.X, op=OP.add)

                # all of apre[tcnk] done: transpose [t, b] -> [b, t] halves
                aps = ftps.tile([128, 4, 128], fp32, tag="aps")
                for h in range(NH):
                    nc.tensor.transpose(
                        aps[:, h, :],
                        apre[tcnk][:, h * 128:(h + 1) * 128], ident32)
                for h in range(NH):
                    nc.scalar.copy(
                        apreT[h][:, tcnk * 128:(tcnk + 1) * 128], aps[:, h, :])

            # ---------------- alpha -> scans -> out ----------------
            for h in range(NH):
                alpha = singles.tile([128, TLOC], fp32, tag=f"alpha{h}",
                                     name=f"alpha{h}")
                nc.scalar.activation(alpha, apreT[h], AF.Sigmoid, bias=b2col)
                nc.vector.tensor_scalar(alpha, alpha, 0.01, 0.99,
                                        op0=OP.max, op1=OP.min)
                A_sb = singles.tile([128, TLOC], fp32, tag=f"A{h}",
                                    name=f"A{h}")
                nc.vector.tensor_scalar(A_sb, alpha, -1.0, 1.0,
                                        op0=OP.mult, op1=OP.add)
                Bv = singles.tile([128, TLOC], fp32, tag=f"Bv{h}",
                                  name=f"Bv{h}")
                nc.vector.tensor_mul(Bv, alpha, rT[h])
                z_sb = singles.tile([128, TLOC], fp32, tag=f"z{h}",
                                    name=f"z{h}")
                nc.vector.tensor_tensor_scan(z_sb, A_sb, Bv, 0.0,
                                             op0=OP.mult, op1=OP.add)
                p_sb = singles.tile([128, TLOC], fp32, tag=f"p{h}",
                                    name=f"p{h}")
                nc.vector.tensor_tensor_scan(p_sb, A_sb, ones_sb, 1.0,
                                             op0=OP.mult, op1=OP.mult)
                nc.scalar.dma_start(z_d[h], z_sb)
                nc.scalar.dma_start(p_d[h], p_sb)

    nc.finalize()
    return nc


def _get_program():
    if "nc" not in _CACHE:
        _CACHE["nc"] = _build_program()
    return _CACHE["nc"]


def kernel(r, feat, W1, b1, W2, b2, _run_kwargs=None, _return_results=False):
    from concourse.bass_utils import run_bass_kernel_spmd

    r = np.asarray(r, dtype=np.float32)
    feat16 = np.asarray(feat, dtype=np.float16)
    W1 = np.asarray(W1, dtype=np.float16)
    b1rep = np.ascontiguousarray(np.broadcast_to(
        np.asarray(b1, dtype=np.float32).reshape(1, 1, HID), (128, 32, HID)))
    w2rep = np.ascontiguousarray(np.broadcast_to(
        np.asarray(W2, dtype=np.float32).reshape(1, 1, HID), (128, 32, HID)))
    b2col = np.ascontiguousarray(np.broadcast_to(
        np.asarray(b2, dtype=np.float32).reshape(1, 1), (128, 1)))

    nc = _get_program()
    in_maps = []
    for c in range(NCORES):
        in_maps.append({
            "feat": np.ascontiguousarray(feat16[c * TLOC:(c + 1) * TLOC]),
            "rt": np.ascontiguousarray(r[c * TLOC:(c + 1) * TLOC, :, 0].T),
            "w1": W1, "b1rep": b1rep, "w2rep": w2rep, "b2col": b2col,
        })

    kw = _run_kwargs or {}
    res = run_bass_kernel_spmd(nc, in_maps, core_ids=list(range(NCORES)), **kw)

    # host stitch: y = z + P*carry per slab, carry chain across slabs
    # z/p layout: [h, p, t] with b = h*128 + p (contiguous halves)
    y = np.empty((T, B), dtype=np.float32)
    carry = r[0, :, 0].astype(np.float32)
    for c in range(NCORES):
        zc = res.results[c]["z"].reshape(B, TLOC).T
        pc = res.results[c]["p"].reshape(B, TLOC).T
        y_slab = zc + pc * carry[None, :]
        carry = y_slab[-1]
        y[c * TLOC:(c + 1) * TLOC] = y_slab
    out = y[:, :, None]
    if _return_results:
        return out, res
    return out


# revision 23
# speedup vs baseline: 1.0360x; 1.0146x over previous
"""EMA head kernel for Trainium2 (Bass/Tile), 8 NeuronCores.

Problem: alpha = clip(sigmoid(MLP(feat)), 0.01, 0.99) per (t, b);
         y[0] = r[0]; y[t] = (1-alpha[t])*y[t-1] + alpha[t]*r[t].

Sharding: time dim T=4096 split into 8 slabs of 512 (all B=256 per core).
Each core computes, for its slab, the local affine-scan pieces
    z[t] = A[t]*z[t-1] + Bv[t]   (z[-1] = 0),   A = 1-alpha, Bv = alpha*r
    P[t] = A[t]*P[t-1]           (P[-1] = 1)
and the host stitches slabs with   y = z + P * carry,  carry' = y[-1].
carry_0 = r[0] reproduces y[0] = r[0] exactly: a*r + (1-a)*r = r.

v9: feat pre-cast to fp16 on host (the MLP runs in fp16 anyway),
halving HBM traffic, loaded via the SYNC HWDGE queue exclusively
(constants and r ride the SCALAR HWDGE queue so feat prefetch is never
blocked), TIME on partitions ([t=128, b=64 * f=128] tiles, 16 KB
contiguous per partition line).  Per batch element: PE transpose
[t,f] -> [f,t] (8 per 2 KB PSUM bank), copy PSUM->SBUF (DVE/ACT
alternating, plain fp16), matmul lhsT=ftT rhs=W1 -> h [t, 16]
collected 32 b-slots per PSUM bank, drain +b1 (DVE, fused PSUM read) /
relu (ACT) / *W2 (GPSIMD) / reduce (DVE) -> apre [t=128, b].  apre is
PE-transposed to [b=128, t] per t-chunk, then sigmoid/clip/A/Bv and
the two tensor_tensor_scans run whole-row at the end.  r arrives
pre-transposed [b, t] from the host.
"""

import numpy as np

T, B, FEAT, HID = 4096, 256, 128, 16
NCORES = 8
TLOC = T // NCORES  # 512
NH = 2              # batch halves of 128 (contiguous: b = h*128 + p)
NTC = TLOC // 128   # 4 t-chunks of 128 partitions
BQ = 64             # batch elems per feat DMA (16 KB/partition chunk)
NBQ = B // BQ       # 4

_CACHE = {}


def _build_program():
    import concourse.bacc as bacc
    import concourse.bass as bass
    import concourse.tile as tile
    from concourse import mybir
    from concourse.masks import make_identity

    fp32 = mybir.dt.float32
    fp16 = mybir.dt.float16
    AF = mybir.ActivationFunctionType
    OP = mybir.AluOpType

    nc = bacc.Bacc("TRN2", target_bir_lowering=False, debug=False,
                   num_devices=NCORES)

    feat_d = nc.dram_tensor("feat", [TLOC, B, FEAT], fp16, kind="ExternalInput")
    rt_d = nc.dram_tensor("rt", [B, TLOC], fp32, kind="ExternalInput")
    w1_d = nc.dram_tensor("w1", [FEAT, HID], fp16, kind="ExternalInput")
    b1_d = nc.dram_tensor("b1rep", [128, 32, HID], fp32, kind="ExternalInput")
    w2_d = nc.dram_tensor("w2rep", [128, 32, HID], fp32, kind="ExternalInput")
    b2_d = nc.dram_tensor("b2col", [128, 1], fp32, kind="ExternalInput")
    z_d = nc.dram_tensor("z", [NH, 128, TLOC], fp32, kind="ExternalOutput")
    p_d = nc.dram_tensor("p", [NH, 128, TLOC], fp32, kind="ExternalOutput")

    with tile.TileContext(nc) as tc:
        with (
            tc.tile_pool(name="singles", bufs=1) as singles,
            tc.tile_pool(name="featin", bufs=3) as featin,
            tc.tile_pool(name="ftps", bufs=3, space="PSUM") as ftps,
            tc.tile_pool(name="apsps", bufs=1, space="PSUM") as apsps,
            tc.tile_pool(name="hps", bufs=4, space="PSUM") as hps,
            tc.tile_pool(name="ftsb", bufs=4) as ftsb,
            tc.tile_pool(name="hwork", bufs=2) as hwork,
        ):
            # ------- constants (scalar HWDGE queue; sync = feat only) -------
            ident = singles.tile([128, 128], fp16)
            make_identity(nc, ident)
            ident32 = singles.tile([128, 128], fp32)
            make_identity(nc, ident32)
            w1_sb = singles.tile([128, HID], fp16)
            nc.scalar.dma_start(w1_sb, w1_d[:, :])
            b1rep = singles.tile([128, 32, HID], fp32)
            nc.scalar.dma_start(b1rep, b1_d[:, :, :])
            w2rep = singles.tile([128, 32, HID], fp32)
            nc.scalar.dma_start(w2rep, w2_d[:, :, :])
            b2col = singles.tile([128, 1], fp32)
            nc.scalar.dma_start(b2col, b2_d[:, :])
            ones_sb = singles.tile([128, TLOC], fp32)
            nc.vector.memset(ones_sb, 1.0)

            # ---- r (pre-transposed on host): rT [b, t] per half ----
            rT = [singles.tile([128, TLOC], fp32, tag=f"rT{h}", name=f"rT{h}")
                  for h in range(NH)]
            for h in range(NH):
                nc.scalar.dma_start(rT[h], rt_d[h * 128:(h + 1) * 128, :])

            # per-tchunk alpha_pre accumulators [128 t, B]
            apre = [singles.tile([128, B], fp32, tag=f"apre{tc_}",
                                 name=f"apre{tc_}")
                    for tc_ in range(NTC)]
            # transposed alpha_pre [128 b, t] per half
            apreT = [singles.tile([128, TLOC], fp32, tag=f"apreT{h}",
                                  name=f"apreT{h}")
                     for h in range(NH)]

            # ---------------- main feat pipeline ----------------
            copy_parity = 0
            for tcnk in range(NTC):
                chunks = [(0, 64), (64, 64), (128, 64), (192, 64)]
                for cb0, cbn in chunks:
                    ft = featin.tile([128, BQ * FEAT], fp16, tag="ft")
                    nc.sync.dma_start(
                        ft[:, :cbn * FEAT],
                        feat_d[tcnk * 128:(tcnk + 1) * 128,
                               cb0:cb0 + cbn, :].rearrange(
                                   "t b f -> t (b f)"))

                    for g in range(cbn // 32):  # 32-b granule of this chunk
                        hbank = hps.tile([128, 32, HID], fp32, tag="hbank")
                        for q in range(0, 32, 8):
                            ftp = ftps.tile([128, 8, 128], fp16, tag="ftp16")
                            for s in range(8):
                                bl = g * 32 + q + s
                                nc.tensor.transpose(
                                    ftp[:, s, :],
                                    ft[:, bl * FEAT:(bl + 1) * FEAT], ident)
                            fts = ftsb.tile([128, 8, 128], fp16, tag="fts")
                            if copy_parity == 0:
                                nc.vector.tensor_copy(fts, ftp)
                            else:
                                nc.scalar.copy(fts, ftp)
                            copy_parity ^= 1
                            for s in range(8):
                                nc.tensor.matmul(hbank[:, q + s, :],
                                                 fts[:, s, :], w1_sb)

                        # drain this 32-b bank -> apre columns
                        b0 = cb0 + g * 32
                        hb = hwork.tile([128, 32, HID], fp32, tag="hb")
                        nc.vector.tensor_add(hb, hbank, b1rep)
                        hrelu = hwork.tile([128, 32, HID], fp32, tag="hrelu")
                        nc.scalar.activation(hrelu, hb, AF.Relu)
                        hw = hwork.tile([128, 32, HID], fp32, tag="hw")
                        nc.gpsimd.tensor_mul(hw, hrelu, w2rep)
                        nc.vector.tensor_reduce(
                            apre[tcnk][:, b0:b0 + 32],
                            hw, axis=mybir.AxisListType.X, op=OP.add)

                # all of apre[tcnk] done: transpose [t, b] -> [b, t] halves
                aps = apsps.tile([128, 4, 128], fp32, tag="aps")
                for h in range(NH):
                    nc.tensor.transpose(
                        aps[:, h, :],
                        apre[tcnk][:, h * 128:(h + 1) * 128], ident32)
                for h in range(NH):
                    nc.scalar.copy(
                        apreT[h][:, tcnk * 128:(tcnk + 1) * 128], aps[:, h, :])

            # ---------------- alpha -> scans -> out ----------------
            for h in range(NH):
                alpha = singles.tile([128, TLOC], fp32, tag=f"alpha{h}",
                                     name=f"alpha{h}")
                nc.scalar.activation(alpha, apreT[h], AF.Sigmoid, bias=b2col)
                nc.vector.tensor_scalar(alpha, alpha, 0.01, 0.99,
                                        op0=OP.max, op1=OP.min)
                A_sb = singles.tile([128, TLOC], fp32, tag=f"A{h}",
                                    name=f"A{h}")
                nc.vector.tensor_scalar(A_sb, alpha, -1.0, 1.0,
                                        op0=OP.mult, op1=OP.add)
                Bv = singles.tile([128, TLOC], fp32, tag=f"Bv{h}",
                                  name=f"Bv{h}")
                nc.vector.tensor_mul(Bv, alpha, rT[h])
                z_sb = singles.tile([128, TLOC], fp32, tag=f"z{h}",
                                    name=f"z{h}")
                nc.vector.tensor_tensor_scan(z_sb, A_sb, Bv, 0.0,
                                             op0=OP.mult, op1=OP.add)
                p_sb = singles.tile([128, TLOC], fp32, tag=f"p{h}",
                                    name=f"p{h}")
                nc.vector.tensor_tensor_scan(p_sb, A_sb, ones_sb, 1.0,
                                             op0=OP.mult, op1=OP.mult)
                nc.scalar.dma_start(z_d[h], z_sb)
                nc.scalar.dma_start(p_d[h], p_sb)

    nc.finalize()
    return nc


def _get_program():
    if "nc" not in _CACHE:
        _CACHE["nc"] = _build_program()
    return _CACHE["nc"]


def kernel(r, feat, W1, b1, W2, b2, _run_kwargs=None, _return_results=False):
    from concourse.bass_utils import run_bass_kernel_spmd

    r = np.asarray(r, dtype=np.float32)
    feat16 = np.asarray(feat, dtype=np.float16)
    W1 = np.asarray(W1, dtype=np.float16)
    b1rep = np.ascontiguousarray(np.broadcast_to(
        np.asarray(b1, dtype=np.float32).reshape(1, 1, HID), (128, 32, HID)))
    w2rep = np.ascontiguousarray(np.broadcast_to(
        np.asarray(W2, dtype=np.float32).reshape(1, 1, HID), (128, 32, HID)))
    b2col = np.ascontiguousarray(np.broadcast_to(
        np.asarray(b2, dtype=np.float32).reshape(1, 1), (128, 1)))

    nc = _get_program()
    in_maps = []
    for c in range(NCORES):
        in_maps.append({
            "feat": np.ascontiguousarray(feat16[c * TLOC:(c + 1) * TLOC]),
            "rt": np.ascontiguousarray(r[c * TLOC:(c + 1) * TLOC, :, 0].T),
            "w1": W1, "b1rep": b1rep, "w2rep": w2rep, "b2col": b2col,
        })

    kw = _run_kwargs or {}
    res = run_bass_kernel_spmd(nc, in_maps, core_ids=list(range(NCORES)), **kw)

    # host stitch: y = z + P*carry per slab, carry chain across slabs
    # z/p layout: [h, p, t] with b = h*128 + p (contiguous halves)
    y = np.empty((T, B), dtype=np.float32)
    carry = r[0, :, 0].astype(np.float32)
    for c in range(NCORES):
        zc = res.results[c]["z"].reshape(B, TLOC).T
        pc = res.results[c]["p"].reshape(B, TLOC).T
        y_slab = zc + pc * carry[None, :]
        carry = y_slab[-1]
        y[c * TLOC:(c + 1) * TLOC] = y_slab
    out = y[:, :, None]
    if _return_results:
        return out, res
    return out
